# revision 1
# baseline (speedup 1.0000x reference)
"""Trainium2 Bass kernel for nn_EnhancedTransformerBlock (8-core Megatron TP).

Layouts: everything on-device is TRANSPOSED ([feature, seq]) so all matmul
contractions sit on the partition dim and all weights load in natural
orientation. Quantized int8 values are carried as bf16 (exact for |v|<=127)
so the big matmuls run at 1 cycle/row. Sharding: QKV/MLP-up column-parallel,
WO/MLP-down row-parallel with ReduceScatter; activations quantized with
global scales via tiny AllReduce(max) collectives.
"""
import numpy as np

import concourse.bass as bass
import concourse.mybir as mybir
import concourse.tile as tile
from concourse import bacc, bass_isa
from concourse.bass_utils import run_bass_kernel_spmd
from concourse.masks import make_identity
from concourse.tile import add_dep_helper

P = 128
S = 1024          # sequence
DM = 4096         # d_model
NCORE = 8
HL = 8            # heads per core
DHL = 512         # local head-dim cols (HL*64)
MLPL = 2048       # local mlp cols
ST = S // P       # 8 seq tiles
FT = DM // P      # 32 model-dim tiles
FTL = DHL // P    # 4 local tiles
MT = MLPL // P    # 16 local mlp tiles
TBL = 2079        # extended rel-pos table length (pre-shifted per partition)
SF = float(np.sqrt(64.0) * 1024.0 ** 0.25)
MAGIC = 12582912.0  # 1.5*2^23: x+M-M == rint(x) for |x|<2^22

F32 = mybir.dt.float32
BF16 = mybir.dt.bfloat16
AL = mybir.AluOpType
AF = mybir.ActivationFunctionType
AX = mybir.AxisListType
RG = [list(range(NCORE))]

_cache = {}
last_results = None


class _Stop(Exception):
    pass


def _phase_limit():
    import os
    v = os.environ.get("KERNEL_PHASE")
    return int(v) if v else 99


def _diag_src(dram_tile, h, q0):
    """AP reading reversed table ([HL, TBL]) as bias[i, j] = tr[h, 1023 - q0 - i + j].

    Inner (j) stride +1 keeps each partition's 4KB read contiguous; the -1
    partition stride only shifts per-partition descriptor bases.
    """
    src = dram_tile[:, :].copy()
    v = src.ap
    v[0] = (-1, P)
    v[1] = (1, S)
    src.ap = v
    src.offset = h * TBL + (TBL - 1) - 1055 - q0
    return src


def _diag_src2(dram_tile, h0, q0):
    """Two heads' bias blocks in one DMA: [P, 2, S]; all strides positive.

    dram_tile is [HL*P, TBL] with row (h*P+p) holding U[2078 + p - m] so that
    element (p, hi, j) = U[1055 + q0 + p - j] of head h0+hi.
    """
    src = dram_tile[:, :].copy()
    v = src.ap
    v[0] = (TBL, P)
    v[1] = (P * TBL, 2)
    v.append((1, S))
    src.ap = v
    src.offset = h0 * P * TBL + (1023 - q0)
    return src


def _build(dbg=False, phase=99):
    nc = bacc.Bacc("TRN2", target_bir_lowering=False, debug=False, num_devices=NCORE)

    io = {"_dbg": dbg}
    def di(name, shape):
        io[name] = nc.dram_tensor(name, shape, F32, kind="ExternalInput")
    di("xt", [DHL, S])
    di("sc1", [DHL]); di("sc2", [DHL])
    di("wq", [DM, DHL]); di("wk", [DM, DHL]); di("wv", [DM, DHL])
    di("bq", [DHL]); di("bk", [DHL]); di("bv", [DHL])
    di("tbl", [HL * P, TBL])
    di("wo", [DHL, DM]); di("bo", [DHL])
    di("w1", [DM, MLPL]); di("b1", [MLPL])
    di("w2", [MLPL, DM]); di("b2", [DHL])
    io["out"] = nc.dram_tensor("out", [DHL, S], F32, kind="ExternalOutput")
    if dbg:
        io["dbg_xq"] = nc.dram_tensor("dbg_xq", [DM, S], BF16, kind="ExternalOutput")
        io["dbg_q"] = nc.dram_tensor("dbg_q", [DHL, S], F32, kind="ExternalOutput")
        io["dbg_k"] = nc.dram_tensor("dbg_k", [DHL, S], F32, kind="ExternalOutput")
        io["dbg_v"] = nc.dram_tensor("dbg_v", [S, DHL], F32, kind="ExternalOutput")
        io["dbg_bias"] = nc.dram_tensor("dbg_bias", [P, S], F32, kind="ExternalOutput")
        io["dbg_ao"] = nc.dram_tensor("dbg_ao", [DHL, S], F32, kind="ExternalOutput")
        io["dbg_x2"] = nc.dram_tensor("dbg_x2", [DHL, S], F32, kind="ExternalOutput")
        io["dbg_h"] = nc.dram_tensor("dbg_h", [MLPL, S], F32, kind="ExternalOutput")
        io["dbg_sc"] = nc.dram_tensor("dbg_sc", [1, 16], F32, kind="ExternalOutput")

    io["_phase"] = phase
    with tile.TileContext(nc) as tc:
        _body(nc, tc, io)
    nc.compile()
    return nc


def _body(nc, tc, io):
    xt, sc1, sc2 = io["xt"], io["sc1"], io["sc2"]
    wq, wk, wv = io["wq"], io["wk"], io["wv"]
    bq, bk, bv, tbl = io["bq"], io["bk"], io["bv"], io["tbl"]
    wo, bo, w1, b1, w2, b2 = io["wo"], io["bo"], io["w1"], io["b1"], io["w2"], io["b2"]
    out = io["out"]

    from contextlib import ExitStack
    top = ExitStack()
    const = top.enter_context(tc.tile_pool(name="const", bufs=1))
    dram = top.enter_context(tc.tile_pool(name="dram", bufs=1, space="DRAM"))

    ident = const.tile([P, P], F32)
    make_identity(nc, ident)
    ones = const.tile([P, 1], F32)
    nc.vector.memset(ones[:, :], 1.0)

    def load_vec(dr, n_tiles, name):
        t = const.tile([P, n_tiles], F32, name=name)
        nc.sync.dma_start(t[:, :], dr[:].rearrange("(o p) -> p o", p=P))
        return t

    sc1_sb = load_vec(sc1, FTL, "sc1_sb")
    sc2_sb = load_vec(sc2, FTL, "sc2_sb")
    bq_sb = load_vec(bq, FTL, "bq_sb")
    bk_sb = load_vec(bk, FTL, "bk_sb")
    bo_sb = load_vec(bo, FTL, "bo_sb")
    b1_sb = load_vec(b1, MT, "b1_sb")
    b2_sb = load_vec(b2, FTL, "b2_sb")

    def bc(src11, name, ch=P):
        t = const.tile([ch, 1], F32, name=name)
        nc.gpsimd.partition_broadcast(t[:, :], src11, channels=ch)
        return t

    def quant_scale(mx11, name):
        """sx = mx/127 + 1e-8 ; returns (sx [1,1] AP, inv_sx_bc [P,1] tile)."""
        sx = const.tile([1, 1], F32, name=name + "_sx")
        nc.vector.tensor_scalar(sx[:, :], mx11, 1.0 / 127.0, 1e-8, AL.mult, AL.add)
        inv = const.tile([1, 1], F32, name=name + "_inv")
        nc.vector.reciprocal(inv[:, :], sx[:, :])
        return sx, bc(inv[:, :], name + "_invbc")

    def smul(a11, b11, name):
        t = const.tile([1, 1], F32, name=name)
        nc.vector.tensor_tensor(t[:, :], a11, b11, AL.mult)
        return t

    def armaxN(vals, tag):
        """AllReduce-max a list of [1,1] APs; returns list of [1,1] AP slices."""
        n = len(vals)
        loc = const.tile([1, n], F32, name=f"arl_{tag}")
        for i, v in enumerate(vals):
            nc.vector.tensor_copy(loc[:, i:i + 1], v)
        ar_in = dram.tile([1, n], F32, name=f"arin_{tag}")
        ar_out = dram.tile([1, n], F32, addr_space="Shared", name=f"arout_{tag}")
        nc.sync.dma_start(ar_in[:, :], loc[:, :])
        nc.gpsimd.collective_compute("AllReduce", AL.max, replica_groups=RG,
                                     ins=[ar_in[:, :].opt()], outs=[ar_out[:, :].opt()])
        g = const.tile([1, n], F32, name=f"arg_{tag}")
        nc.sync.dma_start(g[:, :], ar_out[:, :])
        return [g[:, i:i + 1] for i in range(n)]

    def weight_absmax(pool, w_dr, kt, chunks, tag):
        """local absmax of [kt*128, chunks*512] weight -> [1,1] AP."""
        run = const.tile([P, 1], F32, name=f"wmr_{tag}")
        first = True
        G = 4 if kt % 4 == 0 else 1
        for k0 in range(0, kt, G):
            for c in range(chunks):
                wt = pool.tile([P, G, 512], F32, tag="wm_ld", name=f"wml_{tag}")
                nc.sync.dma_start(wt[:, :, :],
                                  w_dr[k0 * P:(k0 + G) * P, c * 512:(c + 1) * 512]
                                  .rearrange("(g p) x -> p g x", p=P))
                red = pool.tile([P, 1], F32, tag="wm_red", name=f"wmr2_{tag}")
                nc.vector.tensor_reduce(red[:, :], wt[:, :, :], AX.XY, AL.max,
                                        apply_absolute_value=True)
                if first:
                    nc.vector.tensor_copy(run[:, :], red[:, :])
                    first = False
                else:
                    nc.vector.tensor_tensor(run[:, :], run[:, :], red[:, :], AL.max)
        par = const.tile([P, 1], F32, name=f"wmp_{tag}")
        nc.gpsimd.partition_all_reduce(par[:, :], run[:, :], channels=P,
                                       reduce_op=bass_isa.ReduceOp.absmax)
        return par[:1, :]

    def act_absmax(t_ap, tag):
        red = const.tile([P, 1], F32, name=f"am_{tag}")
        nc.vector.tensor_reduce(red[:, :], t_ap, AX.XY, AL.max, apply_absolute_value=True)
        par = const.tile([P, 1], F32, name=f"amp_{tag}")
        nc.gpsimd.partition_all_reduce(par[:, :], red[:, :], channels=P,
                                       reduce_op=bass_isa.ReduceOp.absmax)
        return par[:1, :]

    def wchunk(pool, w_dr, k0, G, c0, width, inv_bc_, tag):
        """load [128, G, width] fp32 chunk rows (k0..k0+G)*128, quantize -> bf16."""
        wt = pool.tile([P, G, width], F32, tag=f"{tag}_ld", name=f"{tag}_ld")
        nc.sync.dma_start(wt[:, :, :],
                          w_dr[k0 * P:(k0 + G) * P, c0:c0 + width]
                          .rearrange("(g p) c -> p g c", p=P))
        nc.vector.tensor_scalar(wt[:, :, :], wt[:, :, :], inv_bc_[:, :1], MAGIC,
                                AL.mult, AL.add)
        wb = pool.tile([P, G, width], BF16, tag=f"{tag}_b", name=f"{tag}_b")
        nc.vector.tensor_scalar(wb[:, :, :], wt[:, :, :], MAGIC, None, AL.subtract)
        return wb

    # persistent activations (x2p opened first: it outlives xtp)
    x2_pool = top.enter_context(tc.tile_pool(name="x2p", bufs=1))
    x2_sb = x2_pool.tile([P, FTL, S], F32, name="x2_sb")
    xt_done = ExitStack()
    xt_pool = xt_done.enter_context(tc.tile_pool(name="xtp", bufs=1))
    xt_sb = xt_pool.tile([P, FTL, S], F32, name="xt_sb")
    nc.sync.dma_start(xt_sb[:, :, :], xt[:, :].rearrange("(o p) f -> p o f", p=P))

    # =========== norm + quantize + gather (both norm layers) ===========
    def norm_quant(x_sb, sc_sb, tag, extra_maxes=()):
        """x_sb [P, FTL, S] f32 shard -> (xq AG dram [DM, S] bf16, sx, extra globals)."""
        nex = len(extra_maxes)
        with tc.tile_pool(name=f"np_{tag}", bufs=2) as npool, \
             tc.tile_pool(name=f"nk_{tag}", bufs=1) as nkeep, \
             tc.tile_pool(name=f"npps_{tag}", bufs=1, space="PSUM") as pps:
            ssq_ps = pps.tile([1, 2, 512], F32, name=f"ssq_ps_{tag}")
            xs_sb = nkeep.tile([P, FTL, S], F32, name=f"xs_{tag}")
            cm = nkeep.tile([P, S], F32, name=f"cm_{tag}")
            for t in range(FTL):
                sq = npool.tile([P, S], F32, tag="sq", name=f"sq_{tag}")
                nc.scalar.activation(sq[:, :], x_sb[:, t, :], AF.Square)
                for n in range(2):
                    nc.tensor.matmul(ssq_ps[:, n, :], ones[:, :],
                                     sq[:, n * 512:(n + 1) * 512],
                                     start=(t == 0), stop=(t == FTL - 1))
                nc.vector.tensor_scalar(xs_sb[:, t, :], x_sb[:, t, :],
                                        sc_sb[:, t:t + 1], None, AL.mult)
                if t == 0:
                    nc.scalar.activation(cm[:, :], xs_sb[:, t, :], AF.Abs)
                else:
                    ab = npool.tile([P, S], F32, tag="ab", name=f"ab_{tag}")
                    nc.scalar.activation(ab[:, :], xs_sb[:, t, :], AF.Abs)
                    nc.vector.tensor_tensor(cm[:, :], cm[:, :], ab[:, :], AL.max)
            cmr = nkeep.tile([P, S], F32, name=f"cmr_{tag}")
            nc.gpsimd.partition_all_reduce(cmr[:, :], cm[:, :], channels=P,
                                           reduce_op=bass_isa.ReduceOp.absmax)
            ssq_sb = npool.tile([1, S], F32, tag="ssq", name=f"ssq_{tag}")
            nc.scalar.copy(ssq_sb[:, :], ssq_ps[:, :, :].rearrange("p a b -> p (a b)"))
            ssq_in = dram.tile([1, S], F32, name=f"ssq_in_{tag}")
            ssq_out = dram.tile([1, S], F32, addr_space="Shared", name=f"ssq_out_{tag}")
            cm_in = dram.tile([1, S + nex], F32, name=f"cm_in_{tag}")
            cm_out = dram.tile([1, S + nex], F32, addr_space="Shared",
                               name=f"cm_out_{tag}")
            nc.sync.dma_start(ssq_in[:, :], ssq_sb[:, :])
            nc.sync.dma_start(cm_in[:, :S], cmr[:1, :])
            for i, em in enumerate(extra_maxes):
                nc.sync.dma_start(cm_in[:, S + i:S + i + 1], em)
            nc.gpsimd.collective_compute("AllReduce", AL.add, replica_groups=RG,
                                         ins=[ssq_in[:, :].opt()], outs=[ssq_out[:, :].opt()])
            nc.gpsimd.collective_compute("AllReduce", AL.max, replica_groups=RG,
                                         ins=[cm_in[:, :].opt()], outs=[cm_out[:, :].opt()])
            ssq_g = npool.tile([1, S], F32, tag="ssqg", name=f"ssqg_{tag}")
            cm_g = nkeep.tile([1, S + nex], F32, name=f"cmg_{tag}")
            nc.sync.dma_start(ssq_g[:, :], ssq_out[:, :])
            nc.sync.dma_start(cm_g[:, :], cm_out[:, :])
            extra_g = [const.tile([1, 1], F32, name=f"exg_{tag}{i}")
                       for i in range(nex)]
            for i in range(nex):
                nc.vector.tensor_copy(extra_g[i][:, :], cm_g[:, S + i:S + i + 1])
            rstd = npool.tile([1, S], F32, tag="rstd", name=f"rstd_{tag}")
            nc.vector.tensor_scalar(rstd[:, :], ssq_g[:, :], 1.0 / DM, 1e-6, AL.mult, AL.add)
            nc.scalar.activation(rstd[:, :], rstd[:, :], AF.Sqrt)
            nc.vector.reciprocal(rstd[:, :], rstd[:, :])
            rstd_bc = nkeep.tile([P, S], F32, name=f"rstdbc_{tag}")
            nc.gpsimd.partition_broadcast(rstd_bc[:, :], rstd[:, :], channels=P)
            sxv = npool.tile([1, S], F32, tag="sxv", name=f"sxv_{tag}")
            nc.vector.tensor_tensor(sxv[:, :], cm_g[:, :S], rstd[:, :], AL.mult)
            mx = const.tile([1, 1], F32, name=f"mx_{tag}")
            nc.vector.tensor_reduce(mx[:, :], sxv[:, :], AX.X, AL.max)
            sx, inv_bc = quant_scale(mx[:, :], f"sx_{tag}")
            xq_c = nkeep.tile([P, FTL, S], BF16, name=f"xqc_{tag}")
            for t in range(FTL):
                xn = npool.tile([P, S], F32, tag="xn", name=f"xn_{tag}")
                nc.vector.tensor_tensor(xn[:, :], xs_sb[:, t, :], rstd_bc[:, :], AL.mult)
                xnq = npool.tile([P, S], F32, tag="xnq", name=f"xnq_{tag}")
                nc.scalar.mul(xnq[:, :], xn[:, :], inv_bc[:, :1])
                nc.vector.tensor_scalar(xq_c[:, t, :], xnq[:, :], MAGIC, MAGIC,
                                        AL.add, AL.subtract)
            ag_in = dram.tile([DHL, S], BF16, name=f"agin_{tag}")
            ag_out = dram.tile([DM, S], BF16, addr_space="Shared", name=f"agout_{tag}")
            nc.sync.dma_start(ag_in[:, :].rearrange("(o p) f -> p o f", p=P), xq_c[:, :, :])
            nc.gpsimd.collective_compute("AllGather", AL.bypass, replica_groups=RG,
                                         ins=[ag_in[:, :].opt()], outs=[ag_out[:, :].opt()])
        return ag_out, sx, extra_g

    dbg = io["_dbg"]
    _phase = io.get("_phase", 99)
    _stacks = [top, xt_done]

    def _ckpt(n):
        if _phase <= n:
            raise _Stop()

    try:
        # qkv weight absmax first (merged into norm1's colmax AllReduce)
        with tc.tile_pool(name="wmaxp", bufs=3) as wmaxp:
            mq = weight_absmax(wmaxp, wq, FT, 1, "wq")
            mk = weight_absmax(wmaxp, wk, FT, 1, "wk")
            mv = weight_absmax(wmaxp, wv, FT, 1, "wv")

        xq_ag, sx1, (gq, gk, gv) = norm_quant(xt_sb, sc1_sb, "n1",
                                              extra_maxes=[mq, mk, mv])
        if dbg:
            nc.sync.dma_start(io["dbg_xq"][:, :], xq_ag[:, :])
        _ckpt(1)

        # spill targets for q/k/v fp32
        q_dram = dram.tile([DHL, S], F32, name="q_dram")
        k_dram = dram.tile([DHL, S], F32, name="k_dram")
        v_dram = dram.tile([S, DHL], F32, name="v_dram")

        swq, invwq_bc = quant_scale(gq, "swq")
        swk, invwk_bc = quant_scale(gk, "swk")
        swv, invwv_bc = quant_scale(gv, "swv")
        _ckpt(2)
        aq_bc = bc(smul(sx1[:, :], swq[:, :], "aq")[:, :], "aqbc")
        ak_bc = bc(smul(sx1[:, :], swk[:, :], "ak")[:, :], "akbc")
        av_bc = bc(smul(sx1[:, :], swv[:, :], "av")[:, :], "avbc")

        with tc.tile_pool(name="xqall", bufs=1) as xq_pool, \
             tc.tile_pool(name="wld", bufs=2) as wldp, \
             tc.tile_pool(name="qkvev", bufs=2) as qev, \
             tc.tile_pool(name="qkvps", bufs=1, space="PSUM") as qkv_ps:
            xq_all = xq_pool.tile([P, FT, S], BF16, name="xq_all")
            for k0 in range(0, FT, 4):
                nc.sync.dma_start(xq_all[:, k0:k0 + 4, :],
                                  xq_ag[k0 * P:(k0 + 4) * P, :]
                                  .rearrange("(g p) f -> p g f", p=P))

            bv_full = qev.tile([1, DHL], F32, tag="bvrow", name="bv_row")
            nc.sync.dma_start(bv_full[:, :], bv[:].unsqueeze(0))
            bv_bc = qev.tile([P, DHL], F32, tag="bvbc", name="bv_bc")
            nc.gpsimd.partition_broadcast(bv_bc[:, :], bv_full[:, :], channels=P)

            # qT / kT: lhsT = w chunk, rhs = xq_all
            for which, w_dr, invw, alpha, bias_sb, dest in (
                    ("q", wq, invwq_bc, aq_bc, bq_sb, q_dram),
                    ("k", wk, invwk_bc, ak_bc, bk_sb, k_dram)):
                pss = [qkv_ps.tile([P, 512], F32, tag=f"ps{i}", name=f"ps_{which}{i}")
                       for i in range(8)]
                for k0 in range(0, FT, 4):
                    wb = wchunk(wldp, w_dr, k0, 4, 0, DHL, invw, "wqk")
                    for g in range(4):
                        k = k0 + g
                        for m in range(FTL):
                            for n in range(2):
                                nc.tensor.matmul(pss[m * 2 + n][:, :],
                                                 wb[:, g, m * P:(m + 1) * P],
                                                 xq_all[:, k, n * 512:(n + 1) * 512],
                                                 start=(k == 0), stop=(k == FT - 1))
                for m in range(FTL):
                    for n in range(2):
                        ev = qev.tile([P, 512], F32, tag="qkev", name=f"ev_{which}")
                        nc.scalar.activation(ev[:, :], pss[m * 2 + n][:, :], AF.Identity,
                                             bias=bias_sb[:, m:m + 1], scale=alpha[:, :1])
                        nc.sync.dma_start(dest[m * P:(m + 1) * P, n * 512:(n + 1) * 512],
                                          ev[:, :])

            # v: lhsT = xq chunk, rhs = wv chunk
            pss_v = [qkv_ps.tile([P, 512], F32, tag=f"ps{i}", name=f"ps_v{i}")
                     for i in range(8)]
            for k0 in range(0, FT, 4):
                wb = wchunk(wldp, wv, k0, 4, 0, DHL, invwv_bc, "wv")
                for g in range(4):
                    k = k0 + g
                    for m in range(ST):
                        nc.tensor.matmul(pss_v[m][:, :], xq_all[:, k, m * P:(m + 1) * P],
                                         wb[:, g, :],
                                         start=(k == 0), stop=(k == FT - 1))
            for m in range(ST):
                ev = qev.tile([P, DHL], F32, tag="vev", name="vev")
                nc.scalar.mul(ev[:, :], pss_v[m][:, :], av_bc[:, :1])
                ev2 = qev.tile([P, DHL], F32, tag="vev2", name="vev2")
                nc.vector.tensor_tensor(ev2[:, :], ev[:, :], bv_bc[:, :], AL.add)
                nc.sync.dma_start(v_dram[m * P:(m + 1) * P, :], ev2[:, :])
                if m == 0:
                    vmax_run = const.tile([P, 1], F32, name="vmax_run")
                    nc.vector.tensor_reduce(vmax_run[:, :], ev2[:, :], AX.X, AL.max,
                                            apply_absolute_value=True)
                else:
                    red = qev.tile([P, 1], F32, tag="vred", name="vred")
                    nc.vector.tensor_reduce(red[:, :], ev2[:, :], AX.X, AL.max,
                                            apply_absolute_value=True)
                    nc.vector.tensor_tensor(vmax_run[:, :], vmax_run[:, :], red[:, :], AL.max)

        _ckpt(3)
        # reload q/k/v, find maxes, AllReduce, quantize
        aoq_cm = tc.tile_pool(name="aoqall", bufs=1)
        aoq_pool = aoq_cm.__enter__()  # outlives attnkeep; closed after WO matmul
        _stacks.append(aoq_cm)
        mid = ExitStack()  # scope: attention working set (closed before WO matmul)
        _stacks.append(mid)
        attn_keep = mid.enter_context(tc.tile_pool(name="attnkeep", bufs=1))
        qq_sb = attn_keep.tile([P, FTL, S], BF16, name="qq_sb")
        kk_sb = attn_keep.tile([P, FTL, S], BF16, name="kk_sb")
        vq_sb = attn_keep.tile([P, ST, DHL], F32, name="vq_sb")
        ao_sb = attn_keep.tile([P, FTL, S], F32, name="ao_sb")

        with tc.tile_pool(name="qkvq", bufs=3) as qkq, \
             tc.tile_pool(name="qkvf", bufs=1) as qkvf:
            q_sb = qkvf.tile([P, FTL, S], F32, name="q_sb")
            k_sb = qkvf.tile([P, FTL, S], F32, name="k_sb")
            v_sb = qkvf.tile([P, ST, DHL], F32, name="v_sb")
            nc.sync.dma_start(q_sb[:, :, :], q_dram[:, :].rearrange("(o p) f -> p o f", p=P))
            nc.sync.dma_start(k_sb[:, :, :], k_dram[:, :].rearrange("(o p) f -> p o f", p=P))
            nc.sync.dma_start(v_sb[:, :, :], v_dram[:, :].rearrange("(o p) f -> p o f", p=P))
            par_v = const.tile([P, 1], F32, name="vmax_par")
            nc.gpsimd.partition_all_reduce(par_v[:, :], vmax_run[:, :], channels=P,
                                           reduce_op=bass_isa.ReduceOp.absmax)
            if dbg:
                nc.sync.dma_start(io["dbg_q"][:, :].rearrange("(o p) f -> p o f", p=P),
                                  q_sb[:, :, :])
                nc.sync.dma_start(io["dbg_k"][:, :].rearrange("(o p) f -> p o f", p=P),
                                  k_sb[:, :, :])
                nc.sync.dma_start(io["dbg_v"][:, :].rearrange("(o p) f -> p o f", p=P),
                                  v_sb[:, :, :])
            gqa, gka, gva = armaxN([act_absmax(q_sb[:, :, :], "q"),
                                    act_absmax(k_sb[:, :, :], "k"),
                                    par_v[:1, :]], "qkvact")
            sq_s, invq_bc = quant_scale(gqa, "sq")
            sk_s, invk_bc = quant_scale(gka, "sk")
            sv_s, invv_bc = quant_scale(gva, "sv")

            sqk = smul(sq_s[:, :], sk_s[:, :], "sqk")
            alph = const.tile([1, 1], F32, name="alph")
            nc.vector.tensor_scalar(alph[:, :], sqk[:, :], 1.0 / SF, None, AL.mult)
            nalph = const.tile([1, 1], F32, name="nalph")
            nc.vector.tensor_scalar(nalph[:, :], alph[:, :], -1.0, None, AL.mult)
            alph_bc, nalph_bc = bc(alph[:, :], "alphbc"), bc(nalph[:, :], "nalphbc")
            inv_sqk = const.tile([1, 1], F32, name="invsqk")
            nc.vector.reciprocal(inv_sqk[:, :], sqk[:, :])
            inv_sqk_bc = bc(inv_sqk[:, :], "invsqkbc")
            sv_bc = bc(sv_s[:, :], "svbc")

            tbl_dram = dram.tile([HL * P, TBL], F32, name="tbl_dram")
            tblw = []
            for hh in range(HL):
                tbl_sb = qkq.tile([P, TBL], F32, tag="tblsb", name="tbl_sb")
                nc.sync.dma_start(tbl_sb[:, :], tbl[hh * P:(hh + 1) * P, :])
                nc.vector.tensor_scalar(tbl_sb[:, :], tbl_sb[:, :],
                                        inv_sqk_bc[:, :1], None, AL.mult)
                tblw.append(nc.sync.dma_start(tbl_dram[hh * P:(hh + 1) * P, :],
                                              tbl_sb[:, :]))

            for t in range(FTL):
                for src, dst, ibc in ((q_sb, qq_sb, invq_bc), (k_sb, kk_sb, invk_bc)):
                    tmp = qkq.tile([P, S], F32, tag="qkq_tmp", name="qkq_tmp")
                    nc.scalar.mul(tmp[:, :], src[:, t, :], ibc[:, :1])
                    nc.gpsimd.tensor_scalar(dst[:, t, :], tmp[:, :], MAGIC, MAGIC,
                                            AL.add, AL.subtract)
            for m in range(ST):
                tmp = qkq.tile([P, DHL], F32, tag="vq_tmp", name="vq_tmp")
                nc.scalar.mul(tmp[:, :], v_sb[:, m, :], invv_bc[:, :1])
                nc.gpsimd.tensor_scalar(vq_sb[:, m, :], tmp[:, :], MAGIC, MAGIC,
                                        AL.add, AL.subtract)

        # wo/w1/w2 maxes (overlaps attention)
        with tc.tile_pool(name="wmaxp2", bufs=3) as wmaxp2:
            mo = weight_absmax(wmaxp2, wo, FTL, 8, "wo")
            m1 = weight_absmax(wmaxp2, w1, FT, 4, "w1")
            m2 = weight_absmax(wmaxp2, w2, MT, 8, "w2")
        _ckpt(4)
        go, g1, g2 = armaxN([mo, m1, m2], "wdown")
        swo, invwo_bc = quant_scale(go, "swo")
        sw1, invw1_bc = quant_scale(g1, "sw1")
        sw2, invw2_bc = quant_scale(g2, "sw2")

        # ================= phase E: attention =================
        with tc.tile_pool(name="attnsb", bufs=3) as asb, \
             tc.tile_pool(name="attnps", bufs=2, space="PSUM") as aps, \
             tc.tile_pool(name="tpps", bufs=2, space="PSUM") as tpps, \
             tc.tile_pool(name="avps", bufs=2, space="PSUM") as avps:
            for hp in range(HL // 2):
                for t in range(ST):
                    attnT = {}
                    bias2 = asb.tile([P, 2, S], F32, tag="bias", name="bias")
                    dinst = nc.sync.dma_start(bias2[:, :, :],
                                              _diag_src2(tbl_dram, hp * 2, t * P))
                    if dbg and hp == 0 and t == 0:
                        nc.sync.dma_start(io["dbg_bias"][:, :], bias2[:, 0, :])
                    try:
                        for _w in (tblw[hp * 2], tblw[hp * 2 + 1]):
                            add_dep_helper(dinst.ins, _w.ins, True, "diag after tbl write")
                    except Exception:
                        pass
                    for hi in range(2):
                        h = hp * 2 + hi
                        pb = 64 * hi
                        cont = aps.tile([P, 2, 512], F32, tag="content", name="content")
                        for n in range(2):
                            nc.tensor.matmul(cont[:, n, :],
                                             qq_sb[pb:pb + 64, hp, t * P:(t + 1) * P],
                                             kk_sb[pb:pb + 64, hp, n * 512:(n + 1) * 512],
                                             start=True, stop=True)
                        s_sb = asb.tile([P, S], F32, tag="s", name="s")
                        nc.vector.tensor_tensor(
                            s_sb[:, :], cont[:, :, :].rearrange("p a b -> p (a b)"),
                            bias2[:, hi, :], AL.add)
                        # scores are tiny (|s*alpha| < ~2): skip max-subtraction;
                        # eps-term difference vs reference is O(1e-9)
                        ex = asb.tile([P, S], F32, tag="ex", name="ex")
                        sume = asb.tile([P, 1], F32, tag="sume", name="sume")
                        nc.scalar.activation(ex[:, :], s_sb[:, :], AF.Exp,
                                             scale=alph_bc[:, :1],
                                             accum_out=sume[:, :])
                        rec = asb.tile([P, 1], F32, tag="rec", name="rec")
                        nc.vector.tensor_scalar(rec[:, :], sume[:, :], 1e-6, None, AL.add)
                        nc.vector.reciprocal(rec[:, :], rec[:, :])
                        attn = asb.tile([P, S], F32, tag="attn", name="attn")
                        nc.gpsimd.tensor_scalar(attn[:, :], ex[:, :], rec[:, :], None, AL.mult)
                        aT = asb.tile([P, ST, P], F32, tag=f"attnT{hi}", name=f"attnT{hi}")
                        for ks in range(ST):
                            tp = tpps.tile([P, P], F32, tag="tp", name="tp")
                            nc.tensor.transpose(tp[:, :], attn[:, ks * P:(ks + 1) * P],
                                                ident[:, :])
                            nc.scalar.copy(aT[:, ks, :], tp[:, :])
                        attnT[hi] = aT
                    avp = avps.tile([P, P], F32, tag="av", name="av")
                    for hi in range(2):
                        h = hp * 2 + hi
                        for ks in range(ST):
                            nc.tensor.matmul(avp[64 * hi:64 * (hi + 1), :],
                                             vq_sb[:, ks, h * 64:(h + 1) * 64],
                                             attnT[hi][:, ks, :],
                                             start=(ks == 0), stop=(ks == ST - 1),
                                             tile_position=(0, 64 * hi))
                    nc.scalar.mul(ao_sb[:, hp, t * P:(t + 1) * P], avp[:, :], sv_bc[:, :1])

        _ckpt(5)
        # ================= phase F: quantize ao, WO matmul, RS =================
        if dbg:
            nc.sync.dma_start(io["dbg_ao"][:, :].rearrange("(o p) f -> p o f", p=P),
                              ao_sb[:, :, :])
        (gao,) = armaxN([act_absmax(ao_sb[:, :, :], "ao")], "aomax")
        sao_s, invao_bc = quant_scale(gao, "sao")
        a_o_bc = bc(smul(sao_s[:, :], swo[:, :], "a_o")[:, :], "a_o_bc")
        aoq_sb = aoq_pool.tile([P, FTL, S], BF16, name="aoq_sb")
        with tc.tile_pool(name="aoq_t", bufs=2) as aoqp:
            for t in range(FTL):
                tmp = aoqp.tile([P, S], F32, tag="aoq_tmp", name="aoq_tmp")
                nc.scalar.mul(tmp[:, :], ao_sb[:, t, :], invao_bc[:, :1])
                nc.gpsimd.tensor_scalar(aoq_sb[:, t, :], tmp[:, :], MAGIC, MAGIC,
                                        AL.add, AL.subtract)
        mid.close()  # free qq/kk/vq/ao
        _stacks.remove(mid)

        def big_matmul(w_dr, kt, mt, rhs_view, invw_bc_, out_dram, evict_fn, tag):
            """out[m*128+p, n*512+j] = sum_k w[k*128+kp, m*128+p] * rhs[kp, k, n*512+j]."""
            with tc.tile_pool(name=f"bm_{tag}", bufs=2) as bmp, \
                 tc.tile_pool(name=f"bmev_{tag}", bufs=2) as bev, \
                 tc.tile_pool(name=f"bmps_{tag}", bufs=1, space="PSUM") as bps:
                pss = [bps.tile([P, 2, 512], F32, tag=f"ps{i}", name=f"ps_{tag}{i}")
                       for i in range(4)]
                G = 4
                for mg in range(mt // 4):
                    for k0 in range(0, kt, G):
                        wb = wchunk(bmp, w_dr, k0, G, mg * 512, 512, invw_bc_, f"{tag}w")
                        for g in range(G):
                            k = k0 + g
                            for mi in range(4):
                                for n in range(2):
                                    nc.tensor.matmul(pss[mi][:, n, :],
                                                     wb[:, g, mi * P:(mi + 1) * P],
                                                     rhs_view(k, n),
                                                     start=(k == 0), stop=(k == kt - 1))
                    for mi in range(4):
                        m = mg * 4 + mi
                        ev = bev.tile([P, S], F32, tag="ev", name=f"ev_{tag}")
                        evict_fn(ev, pss[mi], m, bev)
                        nc.sync.dma_start(out_dram[m * P:(m + 1) * P, :], ev[:, :])

        aout_dram = dram.tile([DM, S], F32, name="aout_dram")

        def evict_wo(ev, ps, m, pool):
            nc.scalar.mul(ev[:, :], ps[:, :, :].rearrange("p a b -> p (a b)"), a_o_bc[:, :1])

        big_matmul(wo, FTL, FT, lambda k, n: aoq_sb[:, k, n * 512:(n + 1) * 512],
                   invwo_bc, aout_dram, evict_wo, "wo")
        aoq_cm.__exit__(None, None, None)
        _stacks.remove(aoq_cm)

        aout_rs = dram.tile([DHL, S], F32, name="aout_rs")
        nc.gpsimd.collective_compute("ReduceScatter", AL.add, replica_groups=RG,
                                     ins=[aout_dram[:, :].opt()], outs=[aout_rs[:, :].opt()])

        with tc.tile_pool(name="x2t", bufs=2) as x2t:
            for t in range(FTL):
                tmp = x2t.tile([P, S], F32, tag="x2_tmp", name="x2_tmp")
                nc.sync.dma_start(tmp[:, :], aout_rs[t * P:(t + 1) * P, :])
                nc.vector.tensor_scalar(tmp[:, :], tmp[:, :], bo_sb[:, t:t + 1], None, AL.add)
                nc.vector.tensor_tensor(x2_sb[:, t, :], tmp[:, :], xt_sb[:, t, :], AL.add)
        xt_done.close()  # free xt_sb
        _stacks.remove(xt_done)

        if dbg:
            nc.sync.dma_start(io["dbg_x2"][:, :].rearrange("(o p) f -> p o f", p=P),
                              x2_sb[:, :, :])
        _ckpt(6)

        # ================= phase G: norm2 =================
        x2q_ag, sx2, _ = norm_quant(x2_sb, sc2_sb, "n2")

        _ckpt(7)
        # ================= phase H: MLP up =================
        a1_bc = bc(smul(sx2[:, :], sw1[:, :], "a1")[:, :], "a1_bc")
        h_dram = dram.tile([MLPL, S], F32, name="h_dram")
        hmax_run = const.tile([P, 1], F32, name="hmax_run")
        hm_first = [True]

        def evict_h(ev, ps, m, pool):
            nc.scalar.activation(ev[:, :], ps[:, :, :].rearrange("p a b -> p (a b)"),
                                 AF.Gelu_apprx_tanh, bias=b1_sb[:, m:m + 1], scale=a1_bc[:, :1])
            red = pool.tile([P, 1], F32, tag="hred", name="hred")
            nc.vector.tensor_reduce(red[:, :], ev[:, :], AX.X, AL.max, apply_absolute_value=True)
            if hm_first[0]:
                nc.vector.tensor_copy(hmax_run[:, :], red[:, :])
                hm_first[0] = False
            else:
                nc.vector.tensor_tensor(hmax_run[:, :], hmax_run[:, :], red[:, :], AL.max)

        with tc.tile_pool(name="x2qall", bufs=1) as x2q_pool:
            x2q_all = x2q_pool.tile([P, FT, S], BF16, name="x2q_all")
            for k0 in range(0, FT, 4):
                nc.sync.dma_start(x2q_all[:, k0:k0 + 4, :],
                                  x2q_ag[k0 * P:(k0 + 4) * P, :]
                                  .rearrange("(g p) f -> p g f", p=P))
            big_matmul(w1, FT, MT, lambda k, n: x2q_all[:, k, n * 512:(n + 1) * 512],
                       invw1_bc, h_dram, evict_h, "w1")

        hmax_par = const.tile([P, 1], F32, name="hmax_par")
        nc.gpsimd.partition_all_reduce(hmax_par[:, :], hmax_run[:, :], channels=P,
                                       reduce_op=bass_isa.ReduceOp.absmax)
        (gh,) = armaxN([hmax_par[:1, :]], "hmax")
        sh_s, invh_bc = quant_scale(gh, "sh")
        _ckpt(8)
        a2_bc = bc(smul(sh_s[:, :], sw2[:, :], "a2")[:, :], "a2_bc")

        y_dram = dram.tile([DM, S], F32, name="y_dram")

        def evict_y(ev, ps, m, pool):
            nc.scalar.mul(ev[:, :], ps[:, :, :].rearrange("p a b -> p (a b)"), a2_bc[:, :1])

        with tc.tile_pool(name="hqall", bufs=1) as hq_pool, \
             tc.tile_pool(name="hldp", bufs=2) as hldp:
            hq_sb = hq_pool.tile([P, MT, S], BF16, name="hq_sb")
            for m in range(MT):
                tmp = hldp.tile([P, S], F32, tag="hld", name="hld")
                nc.sync.dma_start(tmp[:, :], h_dram[m * P:(m + 1) * P, :])
                tmp2 = hldp.tile([P, S], F32, tag="hld2", name="hld2")
                nc.scalar.mul(tmp2[:, :], tmp[:, :], invh_bc[:, :1])
                nc.gpsimd.tensor_scalar(hq_sb[:, m, :], tmp2[:, :], MAGIC, MAGIC,
                                        AL.add, AL.subtract)
            big_matmul(w2, MT, FT, lambda k, n: hq_sb[:, k, n * 512:(n + 1) * 512],
                       invw2_bc, y_dram, evict_y, "w2")

        if dbg:
            nc.sync.dma_start(io["dbg_h"][:, :], h_dram[:, :])
            scs = [sx1[:, :], swq[:, :], swk[:, :], swv[:, :], sq_s[:, :], sk_s[:, :],
                   sv_s[:, :], sao_s[:, :], swo[:, :], sx2[:, :], sw1[:, :], gh,
                   sh_s[:, :], sw2[:, :], alph[:, :], inv_sqk[:, :]]
            scv = const.tile([1, 16], F32, name="dbg_scv")
            for i, s in enumerate(scs):
                nc.vector.tensor_copy(scv[:, i:i + 1], s)
            nc.sync.dma_start(io["dbg_sc"][:, :], scv[:, :])

        y_rs = dram.tile([DHL, S], F32, name="y_rs")
        nc.gpsimd.collective_compute("ReduceScatter", AL.add, replica_groups=RG,
                                     ins=[y_dram[:, :].opt()], outs=[y_rs[:, :].opt()])

        with tc.tile_pool(name="fint", bufs=2) as fint:
            for t in range(FTL):
                tmp = fint.tile([P, S], F32, tag="fin", name="fin")
                nc.sync.dma_start(tmp[:, :], y_rs[t * P:(t + 1) * P, :])
                nc.vector.tensor_scalar(tmp[:, :], tmp[:, :], b2_sb[:, t:t + 1], None, AL.add)
                nc.vector.tensor_tensor(tmp[:, :], tmp[:, :], x2_sb[:, t, :], AL.add)
                nc.sync.dma_start(out[t * P:(t + 1) * P, :], tmp[:, :])

    except _Stop:
        pass
    finally:
        for st in list(reversed(_stacks)):
            try:
                if isinstance(st, ExitStack):
                    st.close()
                else:
                    st.__exit__(None, None, None)
            except Exception:
                pass


def _get_nc(dbg=False):
    ph = _phase_limit()
    key = ("nc_dbg" if dbg else "nc") + str(ph)
    if key not in _cache:
        _cache[key] = _build(dbg, ph)
    return _cache[key]


def kernel(inputs, rms1_scale, wq, bq, wk, bk, wv, bv, rel_pos_emb,
           wo, bo, rms2_scale, w1, b1, w2, b2, **_unused):
    global last_results
    f = np.float32
    x = np.ascontiguousarray(np.asarray(inputs, f).reshape(S, DM).T)   # [DM, S]
    wq, wk, wv = (np.asarray(w, f) for w in (wq, wk, wv))
    w1, w2, wo = np.asarray(w1, f), np.asarray(w2, f), np.asarray(wo, f)
    rel = np.asarray(rel_pos_emb, f)                                   # [65, 64]
    # pre-shifted table: TT[h, p, m] = rel[clip((2078 + p - m) - 1023, 0, 64), h]
    shift_idx = np.clip((2078 + np.arange(P)[:, None] - np.arange(TBL)[None, :])
                        - (S - 1), 0, 64)                              # [P, TBL]
    TTfull = rel[shift_idx]                                            # [P, TBL, 64]

    in_maps = []
    for c in range(NCORE):
        cs, ce = c * DHL, (c + 1) * DHL
        ms, me = c * MLPL, (c + 1) * MLPL
        in_maps.append({
            "xt": np.ascontiguousarray(x[cs:ce]),
            "sc1": np.ascontiguousarray(np.asarray(rms1_scale, f)[cs:ce]),
            "sc2": np.ascontiguousarray(np.asarray(rms2_scale, f)[cs:ce]),
            "wq": np.ascontiguousarray(wq[:, cs:ce]),
            "wk": np.ascontiguousarray(wk[:, cs:ce]),
            "wv": np.ascontiguousarray(wv[:, cs:ce]),
            "bq": np.ascontiguousarray(np.asarray(bq, f)[cs:ce]),
            "bk": np.ascontiguousarray(np.asarray(bk, f)[cs:ce]),
            "bv": np.ascontiguousarray(np.asarray(bv, f)[cs:ce]),
            "tbl": np.ascontiguousarray(
                TTfull[:, :, c * HL:(c + 1) * HL].transpose(2, 0, 1).reshape(HL * P, TBL)),
            "wo": np.ascontiguousarray(wo[cs:ce, :]),
            "bo": np.ascontiguousarray(np.asarray(bo, f)[cs:ce]),
            "w1": np.ascontiguousarray(w1[:, ms:me]),
            "b1": np.ascontiguousarray(np.asarray(b1, f)[ms:me]),
            "w2": np.ascontiguousarray(w2[ms:me, :]),
            "b2": np.ascontiguousarray(np.asarray(b2, f)[cs:ce]),
        })

    import os
    dbg = bool(os.environ.get("KERNEL_DEBUG_DUMPS"))
    nc = _get_nc(dbg)
    res = run_bass_kernel_spmd(nc, in_maps, core_ids=list(range(NCORE)))
    last_results = res
    outT = np.concatenate([res.results[c]["out"] for c in range(NCORE)], axis=0)  # [DM, S]
    return np.ascontiguousarray(outT.T).reshape(1, S, DM).astype(np.float32)

